# revision 7
# baseline (speedup 1.0000x reference)
"""Halo-exchange variant: 8 cores = 2 batch x 4 x1-slabs of 12, with a
2-plane boundary exchange between x1-neighbor cores after each layer
instead of an 8-plane recompute halo.  Per layer each core computes
stage A on 14 planes and stage B on 12 (vs 26..14 / 24..12 shrinking
ranges in the recompute version).

Exchange = two pairwise DRAM AllGathers per layer (E: pairs {0,1},{2,3};
O: pairs {1,2},{0,3} — the 0-3 pair is masked off, it only exists so
every core is in a group).  Payload carries BOTH boundaries (low planes
2,3 + high planes 12,13) so the program is SPMD-uniform; receivers
select the needed half with per-core host-built masks and write the
combined halo planes (0,1,14,15) of the next layer's slab.  Edge-core
masks are zero, which re-zeroes the global D1 padding planes.

Compute layout identical to kernel.py: partition row = 3*(x4+1)+c,
banded 78x78 matmuls, 27 taps x 2 groups for stage A, 27+1 for stage B.
"""

import numpy as np
import ml_dtypes

LAYERS = 4
B, C, D1, D2, D3, D4 = 2, 3, 48, 48, 48, 24
NCORES = 8
SLAB = 12
HALO = 2
NPLANES = SLAB + 2 * HALO   # 16 local planes per core
NROWS = 78
PW = 50
NCHUNK = 5
CHUNK_ROWS = [10, 10, 10, 10, 8]
CHUNK_OFF = [0, 10, 20, 30, 40]

BF16 = ml_dtypes.bfloat16

_cached = {}


def _row_of(x4, c):
    return 3 * (x4 + 1) + c


def _build_weights(Wg, bg, W1, b1, W2, b2, Wd, bd):
    Wg = np.asarray(Wg, np.float32)
    W1 = np.asarray(W1, np.float32)
    W2 = np.asarray(W2, np.float32)
    Wd = np.asarray(Wd, np.float32)
    W21 = np.einsum("lome,lmiabc->loiabce", W2[:, :, :, 0, 0, 0, :], W1[..., 0])

    def band(K4):
        out = np.zeros((27, NROWS, NROWS), np.float32)
        for pi in range(27):
            a, b_, c_ = pi // 9, (pi // 3) % 3, pi % 3
            for o in range(D4):
                for e in range(3):
                    for ci in range(3):
                        for co in range(3):
                            out[pi, 3 * (o + e) + ci, 3 * (o + 1) + co] = \
                                K4[co, ci, a, b_, c_, e]
        return out

    # out group dup row order: copy1 (unshifted) first so Wd can read rows
    # [0:78] of tile 0; bnd dup keeps d2-major order (host-built either way)
    D2OFF_O = {1: 0, 0: 78, 2: 156}
    wAo2 = np.zeros((LAYERS, 128, 2, 9, NROWS), np.float32)
    wA2 = np.zeros((LAYERS, 128, 2, 9, NROWS), np.float32)
    wB2 = np.zeros((LAYERS, 128, 2, 9, NROWS), np.float32)
    wD = np.zeros((LAYERS, NROWS, NROWS), np.float32)
    for l in range(LAYERS):
        # all three band-conv groups with the d2 (x2) taps absorbed into
        # the contraction, split into tiles of 128 + 106 rows
        for t9 in range(9):
            a, c_ = t9 // 3, t9 % 3
            for d2 in range(3):
                for o in range(D4):
                    for e in range(3):
                        for ci in range(3):
                            ro = D2OFF_O[d2] + 3 * (o + e) + ci
                            to, rlo = divmod(ro, 128)
                            r = 78 * d2 + 3 * (o + e) + ci
                            t, rl = divmod(r, 128)
                            for co in range(3):
                                wAo2[l, rlo, to, t9, 3 * (o + 1) + co] = \
                                    Wg[l, co, ci, a, d2, c_, e]
                                wA2[l, rl, t, t9, 3 * (o + 1) + co] = \
                                    Wg[l, co, 3 + ci, a, d2, c_, e]
                                wB2[l, rlo, to, t9, 3 * (o + 1) + co] = \
                                    W21[l, co, ci, a, d2, c_, e]
        for o in range(D4):
            for ci in range(3):
                for co in range(3):
                    wD[l, 3 * (o + 1) + ci, 3 * (o + 1) + co] = Wd[l, co, ci, 0, 0, 0, 0]
    return {
        "wAo2": wAo2.astype(BF16),
        "wA2": wA2.astype(BF16),
        "wB2": wB2.astype(BF16),
        "wD": wD.astype(BF16),
    }


def _bias_tables(bg, b1, b2, bd, W2, q):
    """[L, 2, NPLANES, NROWS, 2]: (mask, bias); zero on globally-invalid
    x1 planes and on the x4-halo rows."""
    bg = np.asarray(bg, np.float32)
    b1 = np.asarray(b1, np.float32)
    b2 = np.asarray(b2, np.float32)
    bd = np.asarray(bd, np.float32)
    W2 = np.asarray(W2, np.float32)
    tab = np.zeros((LAYERS, 2, NPLANES, NROWS, 2), np.float32)
    for l in range(LAYERS):
        rowA = np.zeros(NROWS, np.float32)
        rowB = np.zeros(NROWS, np.float32)
        ones = np.zeros(NROWS, np.float32)
        for x4 in range(D4):
            for c in range(3):
                r = 3 * (x4 + 1) + c
                ones[r] = 1.0
                rowA[r] = bg[l, c]
                acc = b2[l, c] + bd[l, c]
                for e in range(3):
                    if 0 <= x4 + e - 1 < D4:
                        acc += float(np.dot(W2[l, c, :, 0, 0, 0, e], b1[l]))
                rowB[r] = acc
        for p in range(NPLANES):
            g = SLAB * q - HALO + p
            if 0 <= g < D1:
                tab[l, 0, p, :, 0] = ones
                tab[l, 0, p, :, 1] = rowA
                tab[l, 1, p, :, 0] = ones
                tab[l, 1, p, :, 1] = rowB
    return tab


def _halo_masks(q):
    """[NROWS, 6] f32: cols mE_lo, mO_lo, mE_hi, mO_hi, 0.0, 0.0
    (per-partition broadcast)."""
    m = np.zeros((NROWS, 6), np.float32)
    m[:, 0] = 1.0 if q in (1, 3) else 0.0
    m[:, 1] = 1.0 if q == 2 else 0.0
    m[:, 2] = 1.0 if q in (0, 2) else 0.0
    m[:, 3] = 1.0 if q == 1 else 0.0
    return m


def _make_slab(vol, q):
    slab = np.zeros((NROWS, NPLANES, PW, PW), np.float32)
    v = vol.transpose(4, 0, 1, 2, 3)  # [x4, c, x1, x2, x3]
    for p in range(NPLANES):
        g = SLAB * q - HALO + p
        if not (0 <= g < D1):
            continue
        for x4 in range(D4):
            r0 = 3 * (x4 + 1)
            slab[r0:r0 + 3, p, 1:49, 1:49] = v[x4, :, g]
    return slab.astype(BF16)


def _make_bnd2(vol, q):
    """bnd slab with host-side d2 (x2-shift) duplication:
    [NPLANES, 2, 128, PW, PW]; dup row 78*d2 + 3*(x4+1) + c holds the
    plane shifted by d2-1 in x2, split into tiles of 128 + 106 rows."""
    base = np.zeros((NROWS, NPLANES, PW, PW), np.float32)
    v = vol.transpose(4, 0, 1, 2, 3)
    for p in range(NPLANES):
        g = SLAB * q - HALO + p
        if not (0 <= g < D1):
            continue
        for x4 in range(D4):
            r0 = 3 * (x4 + 1)
            base[r0:r0 + 3, p, 1:49, 1:49] = v[x4, :, g]
    full = np.zeros((3 * NROWS, NPLANES, PW, PW), np.float32)
    for d2 in range(3):
        lo = max(0, 1 - d2)
        hi = min(PW, PW + 1 - d2)
        full[78 * d2:78 * d2 + 78, :, lo:hi] = base[:, :, lo + d2 - 1:hi + d2 - 1]
    dup = np.zeros((NPLANES, 2, 128, PW, PW), np.float32)
    dup[:, 0] = full[0:128].transpose(1, 0, 2, 3)
    dup[:, 1, 0:106] = full[128:234].transpose(1, 0, 2, 3)
    return dup.astype(BF16)


def _build_program():
    import concourse.bass as bass
    import concourse.mybir as mybir
    import concourse.tile as tile
    from concourse import bacc

    f32 = mybir.dt.float32
    bf16 = mybir.dt.bfloat16

    nc = bacc.Bacc("TRN2", target_bir_lowering=False, debug=False,
                   num_devices=NCORES)

    fsrc = nc.dram_tensor("fsrc", [NROWS, NPLANES, PW, PW], bf16, kind="ExternalInput")
    bnd2 = nc.dram_tensor("bnd2", [NPLANES, 2, 128, PW, PW], bf16, kind="ExternalInput")
    wAo2d = nc.dram_tensor("wAo2d", [LAYERS, 128, 2, 9, NROWS], bf16, kind="ExternalInput")
    wA2d = nc.dram_tensor("wA2d", [LAYERS, 128, 2, 9, NROWS], bf16, kind="ExternalInput")
    wB2d = nc.dram_tensor("wB2d", [LAYERS, 128, 2, 9, NROWS], bf16, kind="ExternalInput")
    wDd = nc.dram_tensor("wDd", [LAYERS, NROWS, NROWS], bf16, kind="ExternalInput")
    btd = nc.dram_tensor("btd", [LAYERS, 2, NPLANES, NROWS, 2], f32, kind="ExternalInput")
    hmd = nc.dram_tensor("hmd", [NROWS, 6], f32, kind="ExternalInput")
    bufA = nc.dram_tensor("bufA", [NROWS, NPLANES, PW, PW], bf16, kind="Internal")
    bufB = nc.dram_tensor("bufB", [NROWS, NPLANES, PW, PW], bf16, kind="Internal")
    # exchange staging: [row, which(0=lo planes 2,3; 1=hi planes 12,13), i, x2, x3]
    cbi = [nc.dram_tensor(f"cbi{l}", [NROWS, 2, 2, PW, PW], bf16, kind="Internal")
           for l in range(LAYERS - 1)]
    cboE = [nc.dram_tensor(f"cboE{l}", [2, NROWS, 2, 2, PW, PW], bf16, kind="Internal")
            for l in range(LAYERS - 1)]
    cboO = [nc.dram_tensor(f"cboO{l}", [2, NROWS, 2, 2, PW, PW], bf16, kind="Internal")
            for l in range(LAYERS - 1)]
    outd = nc.dram_tensor("outd", [NROWS, SLAB, 48, 48], f32, kind="ExternalOutput")

    GROUPS_E = [[0, 1], [2, 3], [4, 5], [6, 7]]
    GROUPS_O = [[0, 3], [1, 2], [4, 7], [5, 6]]

    with tile.TileContext(nc) as tc:
        with (
            tc.tile_pool(name="wpool", bufs=1) as wpool,
            tc.tile_pool(name="spool", bufs=4) as spool,
            tc.tile_pool(name="bpool", bufs=3) as bpool,
            tc.tile_pool(name="tpA", bufs=4) as tpA,
            tc.tile_pool(name="tpLo", bufs=2) as tpLo,
            tc.tile_pool(name="tpHi", bufs=2) as tpHi,
            tc.tile_pool(name="opool", bufs=2) as opool,
            tc.tile_pool(name="fpool", bufs=2) as fpool,
            tc.tile_pool(name="btpool", bufs=4) as btpool,
            tc.tile_pool(name="hmpool", bufs=1) as hmpool,
            tc.tile_pool(name="chpool", bufs=3) as chpool,
            tc.tile_pool(name="ppa", bufs=1, space="PSUM") as ppa,
            tc.tile_pool(name="ppb", bufs=1, space="PSUM") as ppb,
        ):
            def zero_borders(t):
                nc.vector.memset(t[:, 0, :], 0.0)
                nc.vector.memset(t[:, PW - 1, :], 0.0)
                nc.vector.memset(t[:, :, 0], 0.0)
                nc.vector.memset(t[:, :, PW - 1], 0.0)

            COPY = mybir.ActivationFunctionType.Identity
            MUL = mybir.AluOpType.mult
            ADD = mybir.AluOpType.add

            def drain(eng_is_act, dst_ap, src_ap, mask_ap, bias_ap):
                if eng_is_act:
                    nc.scalar.activation(dst_ap, src_ap, COPY,
                                         bias=bias_ap, scale=mask_ap)
                else:
                    nc.vector.tensor_scalar(dst_ap, src_ap, mask_ap, bias_ap,
                                            MUL, ADD)

            hm = hmpool.tile([NROWS, 6], f32, name="hm")
            nc.sync.dma_start(hm[:], hmd.ap()[:])

            for l in range(LAYERS):
                src = [fsrc, bufA, bufB, bufA][l]
                dst = [bufA, bufB, bufA, None][l]
                wao = wpool.tile([128, 2, 9, NROWS], bf16, name=f"wao{l}", tag="wao")
                wa2 = wpool.tile([128, 2, 9, NROWS], bf16, name=f"wa2{l}", tag="wa2")
                wb = wpool.tile([128, 2, 9, NROWS], bf16, name=f"wb{l}", tag="wb")
                wd = wpool.tile([NROWS, NROWS], bf16, name=f"wd{l}", tag="wd")
                nc.sync.dma_start(wao[:], wAo2d.ap()[l])
                nc.sync.dma_start(wa2[:], wA2d.ap()[l])
                nc.sync.dma_start(wb[:], wB2d.ap()[l])
                nc.sync.dma_start(wd[:], wDd.ap()[l])

                final = l == LAYERS - 1
                scache, bcache, tcache = {}, {}, {}

                def load_src(p):
                    if p not in scache:
                        # load-time d2 duplication: copy1 (unshifted) rows
                        # 0..77, copy0 rows 78..155, copy2 rows 156..233;
                        # cols 0/49 of shifted copies stay unread garbage
                        s0 = spool.tile([128, PW, PW], bf16,
                                        name=f"s{l}_{p}_{len(scache)}a", tag="sw")
                        s1 = spool.tile([106, PW, PW], bf16,
                                        name=f"s{l}_{p}_{len(scache)}b", tag="sw2")
                        nc.sync.dma_start(s0[0:78], src.ap()[:, p])
                        nc.sync.dma_start(s0[78:128, 1:50, :],
                                          src.ap()[0:50, p, 0:49, :])
                        nc.sync.dma_start(s1[0:28, 1:50, :],
                                          src.ap()[50:78, p, 0:49, :])
                        nc.sync.dma_start(s1[28:106, 0:49, :],
                                          src.ap()[0:78, p, 1:50, :])
                        scache[p] = (s0, s1)
                    if p not in bcache:
                        b0 = bpool.tile([128, PW, PW], bf16,
                                        name=f"b{l}_{p}_{len(bcache)}a", tag="bw")
                        b1 = bpool.tile([106, PW, PW], bf16,
                                        name=f"b{l}_{p}_{len(bcache)}b", tag="bw2")
                        nc.sync.dma_start(b0[:], bnd2.ap()[p, 0])
                        nc.sync.dma_start(b1[:], bnd2.ap()[p, 1, 0:106])
                        bcache[p] = (b0, b1)

                def stage_a(x):
                    for p in (x - 1, x, x + 1):
                        load_src(p)
                    bta = btpool.tile([NROWS, 2], f32, name=f"bta{l}_{x}", tag="bt")
                    nc.sync.dma_start(bta[:], btd.ap()[l, 0, x])
                    pt = ppa.tile([128, NCHUNK, 512], f32, name=f"pa{l}_{x}", tag="pa")
                    for pi in range(36):
                        grp, idx = divmod(pi, 18)
                        t9, t = divmod(idx, 2)
                        a, c_ = t9 // 3, t9 % 3
                        cache, wt = ((scache, wao) if grp == 0 else (bcache, wa2))
                        rt2 = cache[x + a - 1][t]
                        rows = 128 if t == 0 else 106
                        for k in range(NCHUNK):
                            nr = CHUNK_ROWS[k]
                            nc.tensor.matmul(
                                pt[0:NROWS, k, :48 * nr],
                                wt[0:rows, t, t9, :],
                                rt2[0:rows,
                                    1 + CHUNK_OFF[k]:1 + CHUNK_OFF[k] + nr,
                                    c_:c_ + 48],
                                start=(pi == 0), stop=(pi == 35),
                            )
                    pool, ptag = ((tpLo, "tlo") if x in (3, 4) else
                                  ((tpHi, "thi") if x in (11, 12) else (tpA, "tw")))
                    t0 = pool.tile([128, PW, PW], bf16, name=f"t{l}_{x}a",
                                   tag=ptag)
                    t1 = pool.tile([106, PW, PW], bf16, name=f"t{l}_{x}b",
                                   tag=ptag + "2")
                    tcache[x] = (t0, t1)
                    zero_borders(t0[0:NROWS])
                    for k in range(NCHUNK):
                        nr = CHUNK_ROWS[k]
                        drain(k in (0, 2, 4),
                              t0[0:NROWS,
                                 1 + CHUNK_OFF[k]:1 + CHUNK_OFF[k] + nr, 1:49],
                              pt[0:NROWS, k, :48 * nr],
                              bta[:, 0:1], bta[:, 1:2])
                    # temp1 d2-duplication (SBUF->SBUF; copy1 is rows 0:78)
                    nc.sync.dma_start(t0[78:128, 1:50, :], t0[0:50, 0:49, :])
                    nc.sync.dma_start(t1[0:28, 1:50, :], t0[50:78, 0:49, :])
                    nc.sync.dma_start(t1[28:106, 0:49, :], t0[0:78, 1:50, :])

                def stage_b(y):
                    btb = btpool.tile([NROWS, 2], f32, name=f"btb{l}_{y}", tag="bt")
                    nc.sync.dma_start(btb[:], btd.ap()[l, 1, y])
                    ot = ft = None
                    for half, ks in ((0, (0, 1, 2)), (1, (3, 4))):
                        qt = ppb.tile([128, 3, 512], f32, name=f"pb{l}_{y}_{half}",
                                      tag="pb")
                        for pi in range(19):
                            for k in ks:
                                nr = CHUNK_ROWS[k]
                                kr = k % 3
                                if pi < 18:
                                    t9, t = divmod(pi, 2)
                                    a, c_ = t9 // 3, t9 % 3
                                    rt2 = tcache[y + a - 1][t]
                                    rows = 128 if t == 0 else 106
                                    nc.tensor.matmul(
                                        qt[0:NROWS, kr, :48 * nr],
                                        wb[0:rows, t, t9, :],
                                        rt2[0:rows,
                                            1 + CHUNK_OFF[k]:1 + CHUNK_OFF[k] + nr,
                                            c_:c_ + 48],
                                        start=(pi == 0), stop=False,
                                    )
                                else:
                                    rt = scache[y][0]
                                    nc.tensor.matmul(
                                        qt[0:NROWS, kr, :48 * nr],
                                        wd[:, :],
                                        rt[0:NROWS,
                                           1 + CHUNK_OFF[k]:1 + CHUNK_OFF[k] + nr,
                                           1:49],
                                        start=False, stop=True,
                                    )
                        if not final:
                            if half == 0:
                                ot = opool.tile([NROWS, PW, PW], bf16,
                                                name=f"o{l}_{y}", tag="ow")
                                zero_borders(ot)
                            for k in ks:
                                nr = CHUNK_ROWS[k]
                                drain(k in (0, 2, 4),
                                      ot[:, 1 + CHUNK_OFF[k]:1 + CHUNK_OFF[k] + nr,
                                         1:49],
                                      qt[0:NROWS, k % 3, :48 * nr],
                                      btb[:, 0:1], btb[:, 1:2])
                        else:
                            if half == 0:
                                ft = fpool.tile([NROWS, 48, 48], f32,
                                                name=f"f{y}", tag="fo")
                            for k in ks:
                                nr = CHUNK_ROWS[k]
                                drain(k in (0, 2, 4),
                                      ft[:, CHUNK_OFF[k]:CHUNK_OFF[k] + nr, :],
                                      qt[0:NROWS, k % 3, :48 * nr],
                                      btb[:, 0:1], btb[:, 1:2])
                    if not final:
                        nc.sync.dma_start(dst.ap()[:, y], ot[:])
                        if y in (2, 3):
                            nc.sync.dma_start(cbi[l].ap()[:, 0, y - 2], ot[:])
                        elif y in (12, 13):
                            nc.sync.dma_start(cbi[l].ap()[:, 1, y - 12], ot[:])
                    else:
                        nc.sync.dma_start(outd.ap()[:, y - HALO], ft[:])

                # boundary-first schedule
                for x in (1, 2, 3):
                    stage_a(x)
                stage_b(2)
                stage_a(4)
                stage_b(3)
                for x in (11, 12, 13):
                    stage_a(x)
                stage_b(12)
                stage_a(14)
                stage_b(13)

                if not final:
                    nc.gpsimd.collective_compute(
                        "AllGather", mybir.AluOpType.bypass,
                        replica_groups=GROUPS_E,
                        ins=[cbi[l].ap()[:]], outs=[cboE[l].ap()[:]],
                    )
                    nc.gpsimd.collective_compute(
                        "AllGather", mybir.AluOpType.bypass,
                        replica_groups=GROUPS_O,
                        ins=[cbi[l].ap()[:]], outs=[cboO[l].ap()[:]],
                    )

                # interior
                scache.clear()
                bcache.clear()
                for x in (5, 6, 7, 8, 9, 10):
                    stage_a(x)
                    stage_b(x - 1)
                stage_b(10)
                stage_b(11)

                # combine received halos -> dst planes 0,1,14,15
                if not final:
                    for (slot, part, cme, cmo, dp0) in (
                            (0, 1, 0, 1, 0),    # low halo <- peers' hi planes
                            (1, 0, 2, 3, 14)):  # high halo <- peers' lo planes
                        for i in range(2):
                            tE = chpool.tile([NROWS, PW, PW], bf16,
                                             name=f"hE{l}_{slot}_{i}", tag="ch")
                            tO = chpool.tile([NROWS, PW, PW], bf16,
                                             name=f"hO{l}_{slot}_{i}", tag="ch")
                            nc.sync.dma_start(tE[:], cboE[l].ap()[slot, :, part, i])
                            nc.sync.dma_start(tO[:], cboO[l].ap()[slot, :, part, i])
                            u = chpool.tile([NROWS, PW, PW], bf16,
                                            name=f"hu{l}_{slot}_{i}", tag="ch")
                            nc.vector.tensor_scalar(u[:], tE[:],
                                                    hm[:, cme:cme + 1],
                                                    hm[:, 4:5], MUL, ADD)
                            nc.scalar.activation(tO[:], tO[:], COPY,
                                                 bias=hm[:, 4:5],
                                                 scale=hm[:, cmo:cmo + 1])
                            nc.vector.tensor_tensor(u[:], u[:], tO[:], ADD)
                            nc.sync.dma_start(dst.ap()[:, dp0 + i], u[:])

    nc.compile()
    return nc


def kernel(f, bondary, Wg, bg, W1, b1, W2, b2, Wd, bd):
    from concourse.bass_utils import run_bass_kernel_spmd

    f = np.asarray(f, np.float32)
    bondary = np.asarray(bondary, np.float32)

    if "nc" not in _cached:
        _cached["nc"] = _build_program()
    nc = _cached["nc"]

    w = _build_weights(Wg, bg, W1, b1, W2, b2, Wd, bd)
    in_maps = []
    for core in range(NCORES):
        b, q = core // 4, core % 4
        in_maps.append({
            "fsrc": _make_slab(f[b], q),
            "bnd2": _make_bnd2(bondary[b], q),
            "wAo2d": w["wAo2"],
            "wA2d": w["wA2"],
            "wB2d": w["wB2"],
            "wDd": w["wD"],
            "btd": _bias_tables(bg, b1, b2, bd, W2, q),
            "hmd": _halo_masks(q),
        })

    res = run_bass_kernel_spmd(nc, in_maps, core_ids=list(range(NCORES)))
    _cached["last_res"] = res

    out = np.zeros((B, C, D1, D2, D3, D4), np.float32)
    rows = np.zeros((D4, C), np.int64)
    for x4 in range(D4):
        for c in range(C):
            rows[x4, c] = _row_of(x4, c)
    for core in range(NCORES):
        b, q = core // 4, core % 4
        arr = res.results[core]["outd"]
        sel = arr[rows.reshape(-1)]
        sel = sel.reshape(D4, C, SLAB, 48, 48)
        out[b, :, SLAB * q:SLAB * q + SLAB] = sel.transpose(1, 2, 3, 4, 0)
    return out
